# revision 8
# baseline (speedup 1.0000x reference)
"""GQA attention kernel for 8 trn2 NeuronCores.

Sharding: core = (b, h) with b = core//4 (batch), h = core%4 (kv head).
Each core handles q heads 4h..4h+3 (a contiguous 512-column block of Wq),
its own kv head (128 rows of Wk/Wv), and the matching 512-column slice of
Wo.  Per-core output is a partial y (row-parallel Wo); host sums the 4
partials per batch.

All matmuls run as float32r (full-rate fp32) with contraction dim on
partitions.  Host pre-transposes x and the weight shards so the device
never transposes activations; only vT -> V (16 tiles) uses PE transpose.
"""

import numpy as np

EMB = 2048
N = 2048          # sequence length
HD = 128          # head dim
NHC = 4           # q heads per core
DQ = NHC * HD     # 512: per-core q concat dim
EC = 16           # e chunks of 128
SC = 16           # s chunks of 128
NB = 512          # n block in projection phase
SCALE = 1.0 / np.sqrt(HD)

_NC = None


def _build(reps=1):
    import concourse.bass as bass
    from concourse import bacc
    import concourse.mybir as mybir
    import concourse.tile as tile
    from concourse.bass import ts

    FP32 = mybir.dt.float32
    R = mybir.dt.float32r
    P = 128

    nc = bacc.Bacc("TRN2", target_bir_lowering=False, debug=False, num_devices=8)
    xT = nc.declare_dram_parameter("xT", [EMB, N], R, isOutput=False)
    wqT = nc.declare_dram_parameter("wqT", [EMB, DQ], R, isOutput=False)
    wkT = nc.declare_dram_parameter("wkT", [EMB, HD], R, isOutput=False)
    wvT = nc.declare_dram_parameter("wvT", [EMB, HD], R, isOutput=False)
    woT = nc.declare_dram_parameter("woT", [DQ, EMB], R, isOutput=False)
    iden_d = nc.declare_dram_parameter("iden", [128, 128], R, isOutput=False)
    ones_d = nc.declare_dram_parameter("ones", [128, 1], R, isOutput=False)
    y = nc.declare_dram_parameter("y", [N, EMB], FP32, isOutput=True)

    xT_r = xT[:].rearrange("(c p) n -> p c n", p=P)      # (128, 16, 2048)
    wqT_r = wqT[:].rearrange("(c p) d -> p c d", p=P)    # (128, 16, 512)
    wkT_r = wkT[:].rearrange("(c p) d -> p c d", p=P)    # (128, 16, 128)
    wvT_r = wvT[:].rearrange("(c p) d -> p c d", p=P)
    woT_r = woT[:].rearrange("(c p) e -> p c e", p=P)    # (128, 4, 2048)

    with tile.TileContext(nc) as tc:
      with tc.tile_pool(name="consts", bufs=1) as consts:
        identity = consts.tile([P, P], R, tag="identity")
        nc.sync.dma_start(identity[:], iden_d[:])
        ones = consts.tile([P, 1], R, tag="ones")
        nc.sync.dma_start(ones[:], ones_d[:])

        for rep in range(reps):
          with tc.tile_pool(name=f"persist{rep}", bufs=1) as persist:
            qT = [persist.tile([P, N], R, tag=f"qT{g}", name=f"qT{g}_{rep}")
                  for g in range(NHC)]
            kT = persist.tile([P, N], R, tag="kT", name=f"kT_{rep}")
            V = persist.tile([P, SC, HD], R, tag="V", name=f"V_{rep}")
            OT = [persist.tile([P, N], R, tag=f"OT{g}", name=f"OT{g}_{rep}")
                  for g in range(NHC)]

            # ---------------- Phase A: projections ----------------
            with tc.tile_pool(name=f"wpool{rep}", bufs=1) as wpool, \
                 tc.tile_pool(name=f"xtp{rep}", bufs=2) as xtp, \
                 tc.tile_pool(name=f"vTp{rep}", bufs=1) as vTp, \
                 tc.tile_pool(name=f"psA{rep}", bufs=4, space="PSUM") as psA, \
                 tc.tile_pool(name=f"psT{rep}", bufs=2, space="PSUM") as psT:
                wq = wpool.tile([P, EC, DQ], R, tag="wq", name=f"wq_{rep}")
                wk = wpool.tile([P, EC, HD], R, tag="wk", name=f"wk_{rep}")
                wv = wpool.tile([P, EC, HD], R, tag="wv", name=f"wv_{rep}")
                nc.sync.dma_start(wq[:], wqT_r)
                nc.sync.dma_start(wk[:], wkT_r)
                nc.sync.dma_start(wv[:], wvT_r)
                vT = vTp.tile([P, N], R, tag="vT", name=f"vT_{rep}")

                for nb in range(N // NB):
                    xt = xtp.tile([P, EC, NB], R, tag="xt", name=f"xt_{rep}_{nb}")
                    nsl = ts(nb, NB)
                    nc.sync.dma_start(xt[:, 0:8, :], xT_r[:, 0:8, nsl])
                    nc.sync.dma_start(xt[:, 8:16, :], xT_r[:, 8:16, nsl])
                    # q projections (4 head tiles), then k, then v
                    for t in range(6):
                        ps = psA.tile([P, NB], FP32, tag="psA",
                                      name=f"psA_{rep}_{nb}_{t}")
                        for e in range(EC):
                            if t < 4:
                                lhsT = wq[:, e, ts(t, HD)]
                            elif t == 4:
                                lhsT = wk[:, e, :]
                            else:
                                lhsT = wv[:, e, :]
                            nc.tensor.matmul(
                                ps[:],
                                lhsT,
                                xt[:, e, :],
                                start=(e == 0),
                                stop=(e == EC - 1),
                            )
                        if t < 4:
                            nc.scalar.copy(qT[t][:, nsl], ps[:])
                        elif t == 4:
                            nc.scalar.copy(kT[:, nsl], ps[:])
                        else:
                            nc.scalar.copy(vT[:, nsl], ps[:])
                    # transpose the 4 freshly-written vT s-chunks into V
                    for j in range(nb * 4, nb * 4 + 4):
                        pt = psT.tile([P, P], R, tag="psT",
                                      name=f"psT_{rep}_{j}")
                        nc.tensor.transpose(pt[:], vT[:, ts(j, P)], identity[:])
                        nc.scalar.copy(V[:, j, :], pt[:])

            # ---------------- Phase B: attention ----------------
            with tc.tile_pool(name=f"wop{rep}", bufs=1) as wop:
              wo = wop.tile([P, NHC, EMB], R, tag="wo", name=f"wo_{rep}")
              nc.sync.dma_start(wo[:], woT_r)
              with tc.tile_pool(name=f"esp{rep}", bufs=3) as esp, \
                 tc.tile_pool(name=f"lap{rep}", bufs=2) as lap, \
                 tc.tile_pool(name=f"rp{rep}", bufs=2) as rp, \
                 tc.tile_pool(name=f"rbp{rep}", bufs=2) as rbp, \
                 tc.tile_pool(name=f"psS{rep}", bufs=2, space="PSUM") as psS, \
                 tc.tile_pool(name=f"psO{rep}", bufs=2, space="PSUM") as psO:

                M = 1024  # n-half size
                for g in range(NHC):
                    for m in range(2):
                        msl = ts(m, M)
                        lacc = lap.tile([P, M], R, tag="lacc",
                                        name=f"lacc_{rep}_{g}_{m}")
                        ot_ps = psO.tile([P, M], FP32, tag="psO",
                                         name=f"psO_{rep}_{g}_{m}")
                        for j in range(SC):
                            s_ps = psS.tile([P, M], FP32, tag="psS",
                                            name=f"psS_{rep}_{g}_{m}_{j}")
                            for u in range(2):
                                nc.tensor.matmul(
                                    s_ps[:, ts(u, 512)],
                                    kT[:, ts(j, P)],
                                    qT[g][:, ts(2 * m + u, 512)],
                                    start=True, stop=True,
                                )
                            es = esp.tile([P, M], R, tag="es",
                                          name=f"es_{rep}_{g}_{m}_{j}")
                            nc.scalar.activation(
                                es[:], s_ps[:],
                                mybir.ActivationFunctionType.Exp,
                                scale=float(SCALE),
                            )
                            if j == 0:
                                nc.vector.tensor_copy(lacc[:], es[:])
                            else:
                                nc.vector.tensor_add(lacc[:], lacc[:], es[:])
                            for u in range(2):
                                nc.tensor.matmul(
                                    ot_ps[:, ts(u, 512)],
                                    V[:, j, :],
                                    es[:, ts(u, 512)],
                                    start=(j == 0), stop=(j == SC - 1),
                                )
                        # partition-reduce lacc via ones-matmul -> (1, M)
                        psl = psS.tile([1, M], FP32, tag="psS",
                                       name=f"psl_{rep}_{g}_{m}")
                        for u in range(2):
                            nc.tensor.matmul(
                                psl[:, ts(u, 512)],
                                ones[:, 0:1],
                                lacc[:, ts(u, 512)],
                                start=True, stop=True,
                            )
                        r_t = rp.tile([1, M], FP32, tag="r",
                                      name=f"r_{rep}_{g}_{m}")
                        nc.vector.reciprocal(r_t[:], psl[:])
                        rb = rbp.tile([P, M], FP32, tag="rb",
                                      name=f"rb_{rep}_{g}_{m}")
                        nc.gpsimd.partition_broadcast(rb[:], r_t[:])
                        nc.vector.tensor_mul(OT[g][:, msl], ot_ps[:], rb[:])

              # ---------------- Phase C: output projection ----------------
              with tc.tile_pool(name=f"yep{rep}", bufs=2) as yep, \
                   tc.tile_pool(name=f"psC{rep}", bufs=2, space="PSUM") as psC:
                  for nt in range(N // P):
                      yp = psC.tile([P, EMB], FP32, tag="psC",
                                    name=f"psC_{rep}_{nt}")
                      for g in range(NHC):
                          lhsT = OT[g][:, ts(nt, P)]
                          for ob in range(4):
                              nc.tensor.matmul(
                                  yp[:, ts(ob, 512)],
                                  lhsT,
                                  wo[:, g, ts(ob, 512)],
                                  start=(g == 0), stop=(g == NHC - 1),
                              )
                      ysb = yep.tile([P, EMB], FP32, tag="ysb",
                                     name=f"ysb_{rep}_{nt}")
                      nc.scalar.copy(ysb[:], yp[:])
                      nc.sync.dma_start(y[ts(nt, P), :], ysb[:])

    nc.compile()
    return nc


def _in_maps(x, Wq, Wk, Wv, Wo):
    x = np.ascontiguousarray(np.asarray(x, dtype=np.float32))
    Wq = np.asarray(Wq, dtype=np.float32)
    Wk = np.asarray(Wk, dtype=np.float32)
    Wv = np.asarray(Wv, dtype=np.float32)
    Wo = np.asarray(Wo, dtype=np.float32)
    xTs = [np.ascontiguousarray(x[b].T) for b in range(2)]
    maps = []
    for core in range(8):
        b, h = divmod(core, 4)
        maps.append({
            "xT": xTs[b],
            "wqT": np.ascontiguousarray(Wq[DQ * h:DQ * (h + 1), :].T),
            "wkT": np.ascontiguousarray(Wk[HD * h:HD * (h + 1), :].T),
            "wvT": np.ascontiguousarray(Wv[HD * h:HD * (h + 1), :].T),
            "woT": np.ascontiguousarray(Wo[:, DQ * h:DQ * (h + 1)].T),
            "iden": np.eye(128, dtype=np.float32),
            "ones": np.ones((128, 1), dtype=np.float32),
        })
    return maps


def run(x, Wq, Wk, Wv, Wo, **spmd_kwargs):
    """Build/compile (cached) and run; returns BassKernelResults."""
    global _NC
    if _NC is None:
        _NC = _build()
    from concourse.bass_utils import run_bass_kernel_spmd
    return run_bass_kernel_spmd(_NC, _in_maps(x, Wq, Wk, Wv, Wo),
                                list(range(8)), **spmd_kwargs)


def kernel(x, attn_mask=None, is_causal=None, Wq=None, Wk=None, Wv=None,
           Wo=None, **_ignored):
    res = run(x, Wq, Wk, Wv, Wo)
    y = np.zeros((2, N, EMB), dtype=np.float32)
    for core in range(8):
        y[core // 4] += res.results[core]["y"]
    return y
